# revision 7
# baseline (speedup 1.0000x reference)
"""CPWanSelfAttention on 8 Trainium2 NeuronCores.

Strategy: tensor-parallel over heads (16 heads -> 2 per core), v2.

Key structure vs v1:
  - RMS norm commutes with RoPE (rstd is a per-position scalar), so rope
    runs on the UN-normalized q/k with no AllReduce dependency; rstd_q /
    rstd_k are applied afterwards as per-column multiplies. The norm
    weight is folded into wq/wk/bq/bk on the host (sumsq uses a
    per-partition 1/nw^2 scalar to recover the un-weighted variance).
  - bf16 everywhere off the PE accumulators: DVE gets 2-4x mode, the
    AllGather + och readback halve, and SBUF pressure drops.
  - Emission order keeps PE saturated: QK for all 4 chunks, then the V
    projection fills PE while DVE ropes; attention starts with no stall.
  - exp processes kt-PAIRS ([128,2x512] PSUM -> one ACT op) to halve the
    per-op ACT overhead; softmax denominator is a bf16 binomial tree of
    tile adds on DVE + one ones-matmul reduction per head-chunk.
  - Collective-dependent DMAs (rstd strips, och gather readback) issue
    from the Pool queue so they can't head-of-line-block input streaming
    on the SP queue.
"""

from contextlib import ExitStack

import numpy as np
import concourse.bass as bass
import concourse.mybir as mybir
import concourse.tile as tile
from concourse import bacc
from concourse.bass_utils import run_bass_kernel_spmd

N_CORES = 8
S = 1992
SP = 2048          # seq padded to multiple of 128 (nki flash attention contract)
DIM = 2048
NHEADS = 16
DH = 128
HPC = NHEADS // N_CORES   # heads per core = 2
DC = DH * HPC             # out dims per core = 256
KT = DIM // 128           # 16 contraction tiles
NCH = SP // 512           # 4 seq chunks of 512
EPS = 1e-6

F32 = mybir.dt.float32
F32R = mybir.dt.float32r
BF16 = mybir.dt.bfloat16

AF = mybir.ActivationFunctionType
ALU = mybir.AluOpType

_COMPILED = None


def _build(ag_mode='chunk4', repeat=1, stage=4):
    nc = bacc.Bacc("TRN2", target_bir_lowering=False, debug=False,
                   num_devices=N_CORES)

    # ---- DRAM I/O (per-core shards) ----
    hidT = nc.dram_tensor("hidT", [NCH, 128, KT, 512], BF16, kind="ExternalInput")
    wqT = nc.dram_tensor("wqT", [128, KT, DC], BF16, kind="ExternalInput")
    wkT = nc.dram_tensor("wkT", [128, KT, DC], BF16, kind="ExternalInput")
    wvT = nc.dram_tensor("wvT", [128, KT, DC], BF16, kind="ExternalInput")
    woT = nc.dram_tensor("woT", [128, KT, DC], BF16, kind="ExternalInput")
    cosT = nc.dram_tensor("cosT", [DH, SP], BF16, kind="ExternalInput")  # [c;c]
    sinT = nc.dram_tensor("sinT", [DH, SP], BF16, kind="ExternalInput")  # [-s;s]
    bq = nc.dram_tensor("bq", [HPC, DH], F32, kind="ExternalInput")
    bk = nc.dram_tensor("bk", [HPC, DH], F32, kind="ExternalInput")
    bv = nc.dram_tensor("bv", [1, DC], BF16, kind="ExternalInput")
    bo = nc.dram_tensor("bo", [HPC, DH], F32, kind="ExternalInput")
    inwq = nc.dram_tensor("inwq", [HPC, DH], F32, kind="ExternalInput")  # 1/nw^2
    inwk = nc.dram_tensor("inwk", [HPC, DH], F32, kind="ExternalInput")
    outT = nc.dram_tensor("outT", [DC, S], F32, kind="ExternalOutput")

    rg = [list(range(N_CORES))]
    inv_sqrt_dh = 1.0 / float(np.sqrt(DH))

    def emit(tc, top, rep):
        P = lambda nm: f"{nm}_{rep}"
        const = top.enter_context(tc.tile_pool(name=P("const"), bufs=1))
        pv_pool = top.enter_context(tc.tile_pool(name=P("pv_pool"), bufs=1))
        dram = top.enter_context(tc.tile_pool(name=P("dram"), bufs=1, space="DRAM"))

        ones_col = const.tile([128, 1], BF16)
        nc.vector.memset(ones_col[:], 1.0)
        ones_sq = const.tile([128, 128], BF16)
        nc.vector.memset(ones_sq[:], 1.0)
        ones_row = const.tile([1, 128], BF16)
        nc.vector.memset(ones_row[:], 1.0)
        bq_sb = const.tile([128, HPC], F32)
        bk_sb = const.tile([128, HPC], F32)
        bo_sb = const.tile([128, HPC], F32)
        inq_sb = const.tile([128, HPC], F32)
        ink_sb = const.tile([128, HPC], F32)
        nc.sync.dma_start(bq_sb[:], bq[:].rearrange("h p -> p h"))
        nc.sync.dma_start(bk_sb[:], bk[:].rearrange("h p -> p h"))
        nc.sync.dma_start(bo_sb[:], bo[:].rearrange("h p -> p h"))
        nc.sync.dma_start(inq_sb[:], inwq[:].rearrange("h p -> p h"))
        nc.sync.dma_start(ink_sb[:], inwk[:].rearrange("h p -> p h"))
        bv_sb = const.tile([1, DC], BF16)
        nc.sync.dma_start(bv_sb[:], bv[:])
        eps1 = const.tile([1, 1], F32)
        nc.vector.memset(eps1[:], EPS)

        v_sb = pv_pool.tile([128, SP // 128, DC], BF16)  # [s%128, s-tile, d]
        late = top.enter_context(tc.tile_pool(name=P("late"), bufs=1))
        qT = [late.tile([128, SP], BF16, name=f"qT{h}_{rep}") for h in range(HPC)]
        kTt = [late.tile([128, SP], BF16, name=f"kTt{h}_{rep}") for h in range(HPC)]

        # AllReduce halves over seq: half m covers s in [m*1024,(m+1)*1024);
        # within a half: cols [0:1024] = q sumsq, [1024:2048] = k sumsq
        ar_in = [dram.tile([1, SP], F32, name=f"ar_in{m}_{rep}") for m in range(2)]
        ar_out = [dram.tile([1, SP], F32, addr_space="Shared", name=f"ar_out{m}_{rep}")
                  for m in range(2)]

        with ExitStack() as ph123:
            rawp = ph123.enter_context(tc.tile_pool(name=P("rawp"), bufs=1))
            qraw = [rawp.tile([128, SP], BF16, name=f"qraw{h}_{rep}") for h in range(HPC)]
            kraw = [rawp.tile([128, SP], BF16, name=f"kraw{h}_{rep}") for h in range(HPC)]

            stat = ph123.enter_context(tc.tile_pool(name=P("stat"), bufs=1))
            rbc = [stat.tile([128, SP], BF16, name=f"rbc{i}_{rep}")
                   for i in range(2)]          # 0 = q, 1 = k (rstd broadcast)
            cos_sb = stat.tile([DH, SP], BF16)
            sin_sb = stat.tile([DH, SP], BF16)
            nc.sync.dma_start(cos_sb[:], cosT[:])
            nc.sync.dma_start(sin_sb[:], sinT[:])
            strip = ph123.enter_context(tc.tile_pool(name=P("strip"), bufs=2))
            rwork = ph123.enter_context(tc.tile_pool(name=P("rwork"), bufs=2))
            psA = ph123.enter_context(tc.tile_pool(name=P("psA"), bufs=2, space="PSUM"))

            def rstd_prep(m):
                """AR half m -> rstd broadcast tiles rbc[0]=q, rbc[1]=k.
                Strip reads on Pool (they wait on the AR); math on ACT/DVE."""
                for i in range(2):                      # 0 = q, 1 = k
                    sv = strip.tile([1, 1024], F32, name="sv")
                    nc.gpsimd.dma_start(sv[:], ar_out[m][:, i * 1024:(i + 1) * 1024])
                    nc.scalar.activation(sv[:], sv[:], AF.Sqrt,
                                         bias=eps1[:], scale=1.0 / DIM)
                    r16 = strip.tile([1, 1024], BF16, name="r16")
                    nc.vector.reciprocal(r16[:], sv[:])
                    rdr = dram.tile([1, 1024], BF16, name=f"rdr{i}{m}_{rep}")
                    nc.sync.dma_start(rdr[:], r16[:])
                    nc.sync.dma_start(
                        rbc[i][:, m * 1024:(m + 1) * 1024],
                        rdr[:].partition_broadcast(128))

            def rope_rot(raw, dst, h, m):
                """rotation only (no norm): dst = raw*cos + swap(raw)*sin."""
                sj = slice(m * 1024, (m + 1) * 1024)
                xs = rwork.tile([128, 1024], BF16, name="xs")
                nc.vector.tensor_copy(xs[0:64, :], raw[h][64:128, sj])
                nc.vector.tensor_copy(xs[64:128, :], raw[h][0:64, sj])
                nc.vector.tensor_mul(dst[h][:, sj], raw[h][:, sj], cos_sb[:, sj])
                nc.vector.tensor_mul(xs[:], xs[:], sin_sb[:, sj])
                nc.vector.tensor_add(dst[h][:, sj], dst[h][:, sj], xs[:])

            def rope_mul(dst, i, h, m):
                """apply rstd (per-column broadcast) in place."""
                sj = slice(m * 1024, (m + 1) * 1024)
                nc.vector.tensor_mul(dst[h][:, sj], dst[h][:, sj], rbc[i][:, sj])

            # ---------- phase 1: QK projections + sumsq, ARs per half ------
            with ExitStack() as ph1:
                wpool = ph1.enter_context(tc.tile_pool(name=P("wpool"), bufs=1))
                hid = ph1.enter_context(tc.tile_pool(name=P("hid"), bufs=1))
                wq_sb = wpool.tile([128, KT, DC], BF16)
                wk_sb = wpool.tile([128, KT, DC], BF16)
                wv_sb = wpool.tile([128, KT, DC], BF16)
                nc.sync.dma_start(wq_sb[:], wqT[:])
                nc.sync.dma_start(wk_sb[:], wkT[:])
                nc.sync.dma_start(wv_sb[:], wvT[:])

                hch = []
                for j in range(NCH):
                    sj = slice(j * 512, (j + 1) * 512)
                    hc = hid.tile([128, KT, 512], BF16, name=f"hch{j}")
                    nc.sync.dma_start(hc[:], hidT[j])
                    hch.append(hc)

                    for (wsb, raw, bias) in ((wq_sb, qraw, bq_sb), (wk_sb, kraw, bk_sb)):
                        for h in range(HPC):
                            pq = psA.tile([128, 512], F32, name="pqk")
                            for t in range(KT):
                                nc.tensor.matmul(
                                    pq[:], wsb[:, t, h * DH:(h + 1) * DH],
                                    hc[:, t, :], start=(t == 0), stop=(t == KT - 1))
                            nc.scalar.activation(raw[h][:, sj], pq[:], AF.Identity,
                                                 bias=bias[:, h:h + 1])

                    # partial sum-of-squares (un-weighted: scale by 1/nw^2)
                    for idx, (raw, inv2) in ((0, (qraw, inq_sb)), (1, (kraw, ink_sb))):
                        pss = psA.tile([1, 512], F32, name="pss")
                        for h in range(HPC):
                            sq = rwork.tile([128, 512], BF16, name="sq")
                            nc.vector.scalar_tensor_tensor(
                                sq[:], raw[h][:, sj], inv2[:, h:h + 1],
                                raw[h][:, sj], ALU.mult, ALU.mult)
                            nc.tensor.matmul(pss[:], ones_col[:], sq[:],
                                             start=(h == 0), stop=(h == HPC - 1))
                        ssv = rwork.tile([1, 512], F32, name="ssv")
                        nc.vector.tensor_copy(ssv[:], pss[:])
                        m, off = j // 2, (j % 2) * 512
                        nc.sync.dma_start(
                            ar_in[m][:, idx * 1024 + off: idx * 1024 + off + 512],
                            ssv[:])

                    if (j == 1 or j == NCH - 1) and stage >= 2:
                        m = j // 2
                        if ag_mode != 'nocoll':
                            nc.gpsimd.collective_compute(
                                "AllReduce", ALU.add, replica_groups=rg,
                                ins=[ar_in[m][:].opt()], outs=[ar_out[m][:].opt()])
                        else:
                            nc.gpsimd.dma_start(ar_out[m][:], ar_in[m][:])
                    if j == 1 and stage >= 2:
                        # k rope for half 0 on DVE while PE continues QK
                        rope_rot(kraw, kTt, 0, 0)
                        rope_rot(kraw, kTt, 1, 0)

                # ---- v phase on PE; rope + rstd prep on DVE/ACT/Pool ----
                for j in range(NCH):
                    for st in range(4):
                        gst = j * 4 + st
                        pvp = psA.tile([128, DC], F32, name="pvp")
                        for t in range(KT):
                            nc.tensor.matmul(
                                pvp[:], hch[j][:, t, st * 128:(st + 1) * 128],
                                wv_sb[:, t, :], start=(t == 0), stop=False)
                        nc.tensor.matmul(pvp[:], ones_row[:], bv_sb[:],
                                         start=False, stop=True)
                        if gst == SP // 128 - 1:
                            nc.vector.memset(v_sb[:, gst, :], 0.0)
                            nv = S - (SP // 128 - 1) * 128
                            nc.scalar.activation(v_sb[0:nv, gst, :], pvp[0:nv, :],
                                                 AF.Copy)
                        else:
                            nc.scalar.activation(v_sb[:, gst, :], pvp[:], AF.Copy)

                if stage >= 2:
                    rope_rot(qraw, qT, 0, 0)
                    rope_rot(qraw, qT, 1, 0)
                    rstd_prep(0)
                    for h in range(HPC):
                        rope_mul(kTt, 1, h, 0)
                        rope_mul(qT, 0, h, 0)
                    rope_rot(kraw, kTt, 0, 1)
                    rope_rot(kraw, kTt, 1, 1)
                    rope_rot(qraw, qT, 0, 1)
                    rope_rot(qraw, qT, 1, 1)
                    rstd_prep(1)
                    for h in range(HPC):
                        rope_mul(kTt, 1, h, 1)
                        nc.vector.memset(kTt[h][:, S:SP], 0.0)
                    for h in range(HPC):
                        rope_mul(qT, 0, h, 1)

        if stage < 3:
            return
        # ---------- attention, chunked AllGather, projection ----
        with ExitStack() as ph4:
            aw = ph4.enter_context(tc.tile_pool(name=P("aw"), bufs=2))
            oT = [aw.tile([128, SP], BF16, name=f"oT{h}_{rep}", bufs=1)
                  for h in range(HPC)]
            expp = ph4.enter_context(tc.tile_pool(name=P("expp"), bufs=3))
            denp = ph4.enter_context(tc.tile_pool(name=P("denp"), bufs=2))
            psC = ph4.enter_context(tc.tile_pool(name=P("psC"), bufs=2, space="PSUM"))
            psPV = ph4.enter_context(tc.tile_pool(name=P("psPV"), bufs=2, space="PSUM"))
            psM = ph4.enter_context(tc.tile_pool(name=P("psM"), bufs=2, space="PSUM"))

            wo_sb = aw.tile([128, KT, DC], BF16, bufs=1)
            nc.sync.dma_start(wo_sb[:], woT[:])

            agos = []

            def project(j):
                ago = agos[j]
                och = aw.tile([128, KT, 512], BF16, name="och")
                for q4 in range(4):
                    nc.gpsimd.dma_start(
                        och[:, q4 * 4:(q4 + 1) * 4, :],
                        ago[q4 * 512:(q4 + 1) * 512, :]
                        .rearrange("(t p) s -> p t s", p=128))
                for h in range(HPC):
                    pout = psM.tile([128, 512], F32, name="psm")
                    for t in range(KT):
                        nc.tensor.matmul(
                            pout[:], wo_sb[:, t, h * DH:(h + 1) * DH],
                            och[:, t, :], start=(t == 0), stop=(t == KT - 1))
                    ot = aw.tile([128, 512], F32, name="ot")
                    nc.scalar.activation(ot[:], pout[:], AF.Identity,
                                         bias=bo_sb[:, h:h + 1])
                    w = min(512, S - j * 512)
                    nc.sync.dma_start(
                        outT[h * DH:(h + 1) * DH, j * 512:j * 512 + w],
                        ot[:, 0:w])

            for j in range(NCH):
                sj = slice(j * 512, (j + 1) * 512)
                for h in range(HPC):
                    po = psPV.tile([128, 512], F32, name="pv")
                    # software pipeline: scores one kt-pair ahead of PV
                    pscores = []
                    partials = []   # binomial tree of bf16 exp-tile sums

                    def scores(kp):
                        ps = psC.tile([128, 1024], F32, name="psc")
                        nc.tensor.matmul(
                            ps[:, 0:512], kTt[h][:, kp * 256:kp * 256 + 128],
                            qT[h][:, sj], start=True, stop=True)
                        nc.tensor.matmul(
                            ps[:, 512:1024], kTt[h][:, kp * 256 + 128:kp * 256 + 256],
                            qT[h][:, sj], start=True, stop=True)
                        pscores.append(ps)

                    def exp_pv(kp):
                        ps = pscores[kp]
                        et = expp.tile([128, 1024], BF16, name="et")
                        nc.scalar.activation(et[:], ps[:], AF.Exp,
                                             scale=inv_sqrt_dh)
                        nc.tensor.matmul(
                            po[:], v_sb[:, 2 * kp, h * DH:(h + 1) * DH],
                            et[:, 0:512], start=(kp == 0), stop=False)
                        nc.tensor.matmul(
                            po[:], v_sb[:, 2 * kp + 1, h * DH:(h + 1) * DH],
                            et[:, 512:1024], start=False, stop=(kp == 7))
                        # denominator partials on DVE (bf16 binomial tree)
                        cur, rank = et, 0
                        while partials and partials[-1][1] == rank:
                            prev, _ = partials.pop()
                            dst = denp.tile([128, 1024], BF16, name=f"den{rank}")
                            nc.vector.tensor_add(dst[:], prev[:], cur[:])
                            cur, rank = dst, rank + 1
                        partials.append((cur, rank))

                    scores(0)
                    for kp in range(8):
                        if kp + 1 < 8:
                            scores(kp + 1)
                        exp_pv(kp)
                    acc = partials[0][0]
                    psm = psM.tile([128, 512], F32, name="psm")
                    nc.tensor.matmul(psm[:], ones_sq[:], acc[:, 0:512],
                                     start=True, stop=False)
                    nc.tensor.matmul(psm[:], ones_sq[:], acc[:, 512:1024],
                                     start=False, stop=True)
                    rec = aw.tile([128, 512], F32, name="rec")
                    nc.vector.reciprocal(rec[:], psm[:])
                    nc.vector.tensor_mul(oT[h][:, sj], po[:], rec[:])

                # issue this chunk's AllGather right away; it overlaps the
                # attention of the remaining chunks
                agi = dram.tile([DC, 512], BF16, name=f"agi{j}_{rep}")
                ago = dram.tile([DIM, 512], BF16, addr_space="Shared",
                                name=f"ago{j}_{rep}")
                for h in range(HPC):
                    nc.sync.dma_start(agi[h * DH:(h + 1) * DH, :], oT[h][:, sj])
                if ag_mode == 'chunk4':
                    nc.gpsimd.collective_compute(
                        "AllGather", ALU.bypass, replica_groups=rg,
                        ins=[agi[:].opt()], outs=[ago[:].opt()])
                agos.append(ago)
                if stage >= 4 and j >= 2:
                    project(j - 2)

            if stage >= 4:
                project(NCH - 2)
                project(NCH - 1)

    with tile.TileContext(nc) as tc, \
            nc.allow_low_precision(reason="bf16 softmax path validated vs ref"):
        for rep in range(repeat):
            with ExitStack() as top:
                emit(tc, top, rep)

    nc.compile()
    return nc


def _prep_inputs(hidden_states, freqs_cos, freqs_sin, wq, bq, wk, bk, wv, bv,
                 norm_q_w, norm_k_w, wo, bo):
    """Host-side shard + layout prep. Returns in_maps for 8 cores."""
    f32 = np.float32
    import ml_dtypes
    bf16 = ml_dtypes.bfloat16

    hid = np.asarray(hidden_states)[0].T.astype(f32)
    hidT = np.zeros((DIM, SP), dtype=f32)
    hidT[:, :S] = hid
    # pre-tile to [chunk j, partition p, ktile t, col c]: d = t*128+p, s = j*512+c
    hidT = np.ascontiguousarray(
        hidT.reshape(KT, 128, SP // 512, 512).transpose(2, 1, 0, 3)).astype(bf16)

    def tile_w(wT):                       # [DIM, DC] -> [128, KT, DC]
        return np.ascontiguousarray(
            wT.reshape(KT, 128, DC).transpose(1, 0, 2)).astype(bf16)

    # RoPE tables: c_j[s] = cos[0,s,0,2j], s_j[s] = sin[0,s,0,2j+1]; stack [t;t]
    c = np.asarray(freqs_cos)[0, :, 0, 0::2].astype(f32).T          # [64, S]
    s = np.asarray(freqs_sin)[0, :, 0, 1::2].astype(f32).T          # [64, S]
    cosT = np.zeros((DH, SP), dtype=f32)
    sinT = np.zeros((DH, SP), dtype=f32)
    cosT[0:64, :S] = c
    cosT[64:128, :S] = c
    sinT[0:64, :S] = -s
    sinT[64:128, :S] = s
    cosT = cosT.astype(bf16)
    sinT = sinT.astype(bf16)

    perm = np.concatenate([np.arange(0, DH, 2), np.arange(1, DH, 2)])
    wq = np.asarray(wq)
    wk = np.asarray(wk)
    wv = np.asarray(wv)
    wo = np.asarray(wo)
    bqv = np.asarray(bq)
    bkv = np.asarray(bk)
    bvv = np.asarray(bv)
    bov = np.asarray(bo)
    nq = np.asarray(norm_q_w)
    nk = np.asarray(norm_k_w)

    in_maps = []
    for core in range(N_CORES):
        rows = slice(core * DC, (core + 1) * DC)

        def permuted(mat_rows):                                     # [DC, DIM]
            blocks = [mat_rows[h * DH:(h + 1) * DH][perm] for h in range(HPC)]
            return np.concatenate(blocks, axis=0)

        def permuted_vec(vec_rows):                                 # [HPC, DH]
            blocks = [vec_rows[h * DH:(h + 1) * DH][perm] for h in range(HPC)]
            return np.stack(blocks, axis=0)

        # fold the norm weight into wq/wk and bq/bk (rows scaled by nw)
        nq_p = permuted_vec(nq[rows].astype(f32))                   # [HPC, DH]
        nk_p = permuted_vec(nk[rows].astype(f32))
        wq_c = permuted(wq[rows].astype(f32)) * nq_p.reshape(DC, 1)
        wk_c = permuted(wk[rows].astype(f32)) * nk_p.reshape(DC, 1)
        bq_c = permuted_vec(bqv[rows].astype(f32)) * nq_p
        bk_c = permuted_vec(bkv[rows].astype(f32)) * nk_p

        in_maps.append({
            "hidT": hidT,
            "wqT": tile_w(np.ascontiguousarray(wq_c.T)),
            "wkT": tile_w(np.ascontiguousarray(wk_c.T)),
            "wvT": tile_w(np.ascontiguousarray(wv[rows].astype(f32).T)),
            "woT": tile_w(np.ascontiguousarray(wo[rows].astype(f32).T)),
            "cosT": cosT,
            "sinT": sinT,
            "bq": bq_c,
            "bk": bk_c,
            "bv": bvv[rows].astype(bf16).reshape(1, DC),
            "bo": bov[rows].astype(f32).reshape(HPC, DH),
            "inwq": 1.0 / (nq_p * nq_p),
            "inwk": 1.0 / (nk_p * nk_p),
        })
    return in_maps


def kernel(**inputs):
    global _COMPILED
    if _COMPILED is None:
        _COMPILED = _build()
    nc = _COMPILED
    in_maps = _prep_inputs(**inputs)
    res = run_bass_kernel_spmd(nc, in_maps, core_ids=list(range(N_CORES)))
    out = np.empty((1, S, DIM), dtype=np.float32)
    for core in range(N_CORES):
        out[0, :, core * DC:(core + 1) * DC] = res.results[core]["outT"].T
    return out


# revision 9
# speedup vs baseline: 1.0416x; 1.0416x over previous
"""CPWanSelfAttention on 8 Trainium2 NeuronCores.

Strategy: tensor-parallel over heads (16 heads -> 2 per core), v2.

Key structure vs v1:
  - RMS norm commutes with RoPE (rstd is a per-position scalar), so rope
    runs on the UN-normalized q/k with no AllReduce dependency; rstd_q /
    rstd_k are applied afterwards as per-column multiplies. The norm
    weight is folded into wq/wk/bq/bk on the host (sumsq uses a
    per-partition 1/nw^2 scalar to recover the un-weighted variance).
  - bf16 everywhere off the PE accumulators: DVE gets 2-4x mode, the
    AllGather + och readback halve, and SBUF pressure drops.
  - Emission order keeps PE saturated: QK for all 4 chunks, then the V
    projection fills PE while DVE ropes; attention starts with no stall.
  - exp processes kt-PAIRS ([128,2x512] PSUM -> one ACT op) to halve the
    per-op ACT overhead; softmax denominator is a bf16 binomial tree of
    tile adds on DVE + one ones-matmul reduction per head-chunk.
  - Collective-dependent DMAs (rstd strips, och gather readback) issue
    from the Pool queue so they can't head-of-line-block input streaming
    on the SP queue.
"""

from contextlib import ExitStack

import numpy as np
import concourse.bass as bass
import concourse.mybir as mybir
import concourse.tile as tile
from concourse import bacc
from concourse.bass_utils import run_bass_kernel_spmd

N_CORES = 8
S = 1992
SP = 2048          # seq padded to multiple of 128 (nki flash attention contract)
DIM = 2048
NHEADS = 16
DH = 128
HPC = NHEADS // N_CORES   # heads per core = 2
DC = DH * HPC             # out dims per core = 256
KT = DIM // 128           # 16 contraction tiles
NCH = SP // 512           # 4 seq chunks of 512
EPS = 1e-6

F32 = mybir.dt.float32
F32R = mybir.dt.float32r
BF16 = mybir.dt.bfloat16

AF = mybir.ActivationFunctionType
ALU = mybir.AluOpType

_COMPILED = None


def _build(ag_mode='chunk4', repeat=1, stage=4):
    nc = bacc.Bacc("TRN2", target_bir_lowering=False, debug=False,
                   num_devices=N_CORES)

    # ---- DRAM I/O (per-core shards) ----
    hidT = nc.dram_tensor("hidT", [NCH, 128, KT, 512], BF16, kind="ExternalInput")
    wqT = nc.dram_tensor("wqT", [128, KT, DC], BF16, kind="ExternalInput")
    wkT = nc.dram_tensor("wkT", [128, KT, DC], BF16, kind="ExternalInput")
    wvT = nc.dram_tensor("wvT", [128, KT, DC], BF16, kind="ExternalInput")
    woT = nc.dram_tensor("woT", [128, KT, DC], BF16, kind="ExternalInput")
    cosT = nc.dram_tensor("cosT", [DH, SP], BF16, kind="ExternalInput")  # [c;c]
    sinT = nc.dram_tensor("sinT", [DH, SP], BF16, kind="ExternalInput")  # [-s;s]
    bq = nc.dram_tensor("bq", [HPC, DH], F32, kind="ExternalInput")
    bk = nc.dram_tensor("bk", [HPC, DH], F32, kind="ExternalInput")
    bv = nc.dram_tensor("bv", [1, DC], BF16, kind="ExternalInput")
    bo = nc.dram_tensor("bo", [HPC, DH], F32, kind="ExternalInput")
    inwq = nc.dram_tensor("inwq", [HPC, DH], F32, kind="ExternalInput")  # 1/nw^2
    inwk = nc.dram_tensor("inwk", [HPC, DH], F32, kind="ExternalInput")
    outT = nc.dram_tensor("outT", [DC, S], F32, kind="ExternalOutput")

    rg = [list(range(N_CORES))]
    inv_sqrt_dh = 1.0 / float(np.sqrt(DH))

    def emit(tc, top, rep):
        P = lambda nm: f"{nm}_{rep}"
        const = top.enter_context(tc.tile_pool(name=P("const"), bufs=1))
        pv_pool = top.enter_context(tc.tile_pool(name=P("pv_pool"), bufs=1))
        dram = top.enter_context(tc.tile_pool(name=P("dram"), bufs=1, space="DRAM"))

        ones_col = const.tile([128, 1], BF16)
        nc.vector.memset(ones_col[:], 1.0)
        ones_sq = const.tile([128, 128], BF16)
        nc.vector.memset(ones_sq[:], 1.0)
        ones_row = const.tile([1, 128], BF16)
        nc.vector.memset(ones_row[:], 1.0)
        bq_sb = const.tile([128, HPC], F32)
        bk_sb = const.tile([128, HPC], F32)
        bo_sb = const.tile([128, HPC], F32)
        inq_sb = const.tile([128, HPC], F32)
        ink_sb = const.tile([128, HPC], F32)
        nc.sync.dma_start(bq_sb[:], bq[:].rearrange("h p -> p h"))
        nc.sync.dma_start(bk_sb[:], bk[:].rearrange("h p -> p h"))
        nc.sync.dma_start(bo_sb[:], bo[:].rearrange("h p -> p h"))
        nc.sync.dma_start(inq_sb[:], inwq[:].rearrange("h p -> p h"))
        nc.sync.dma_start(ink_sb[:], inwk[:].rearrange("h p -> p h"))
        bv_sb = const.tile([1, DC], BF16)
        nc.sync.dma_start(bv_sb[:], bv[:])
        eps1 = const.tile([1, 1], F32)
        nc.vector.memset(eps1[:], EPS)

        v_sb = pv_pool.tile([128, SP // 128, DC], BF16)  # [s%128, s-tile, d]
        late = top.enter_context(tc.tile_pool(name=P("late"), bufs=1))
        qT = [late.tile([128, SP], BF16, name=f"qT{h}_{rep}") for h in range(HPC)]
        kTt = [late.tile([128, SP], BF16, name=f"kTt{h}_{rep}") for h in range(HPC)]

        # AllReduce halves over seq: half m covers s in [m*1024,(m+1)*1024);
        # within a half: cols [0:1024] = q sumsq, [1024:2048] = k sumsq
        ar_in = [dram.tile([1, SP], F32, name=f"ar_in{m}_{rep}") for m in range(2)]
        ar_out = [dram.tile([1, SP], F32, addr_space="Shared", name=f"ar_out{m}_{rep}")
                  for m in range(2)]

        with ExitStack() as ph123:
            rawp = ph123.enter_context(tc.tile_pool(name=P("rawp"), bufs=1))
            qraw = [rawp.tile([128, SP], BF16, name=f"qraw{h}_{rep}") for h in range(HPC)]
            kraw = [rawp.tile([128, SP], BF16, name=f"kraw{h}_{rep}") for h in range(HPC)]

            stat = ph123.enter_context(tc.tile_pool(name=P("stat"), bufs=1))
            rbc = [stat.tile([128, SP], F32, name=f"rbc{i}_{rep}")
                   for i in range(2)]          # 0 = q, 1 = k (rstd broadcast)
            cos_sb = stat.tile([DH, SP], BF16)
            sin_sb = stat.tile([DH, SP], BF16)
            nc.sync.dma_start(cos_sb[:], cosT[:])
            nc.sync.dma_start(sin_sb[:], sinT[:])
            strip = ph123.enter_context(tc.tile_pool(name=P("strip"), bufs=2))
            rwork = ph123.enter_context(tc.tile_pool(name=P("rwork"), bufs=2))
            psA = ph123.enter_context(tc.tile_pool(name=P("psA"), bufs=2, space="PSUM"))

            def rstd_prep(m):
                """AR half m -> rstd broadcast tiles rbc[0]=q, rbc[1]=k.
                Strip reads on Pool (they wait on the AR); math on ACT/DVE."""
                for i in range(2):                      # 0 = q, 1 = k
                    sv = strip.tile([1, 1024], F32, name="sv")
                    nc.gpsimd.dma_start(sv[:], ar_out[m][:, i * 1024:(i + 1) * 1024])
                    nc.scalar.activation(sv[:], sv[:], AF.Sqrt,
                                         bias=eps1[:], scale=1.0 / DIM)
                    r16 = strip.tile([1, 1024], F32, name="r16")
                    nc.vector.reciprocal(r16[:], sv[:])
                    rdr = dram.tile([1, 1024], F32, name=f"rdr{i}{m}_{rep}")
                    nc.sync.dma_start(rdr[:], r16[:])
                    nc.sync.dma_start(
                        rbc[i][:, m * 1024:(m + 1) * 1024],
                        rdr[:].partition_broadcast(128))

            def rope_rot(raw, dst, h, m):
                """rotation only (no norm): dst = raw*cos + swap(raw)*sin."""
                sj = slice(m * 1024, (m + 1) * 1024)
                xs = rwork.tile([128, 1024], BF16, name="xs")
                nc.vector.tensor_copy(xs[0:64, :], raw[h][64:128, sj])
                nc.vector.tensor_copy(xs[64:128, :], raw[h][0:64, sj])
                nc.vector.tensor_mul(dst[h][:, sj], raw[h][:, sj], cos_sb[:, sj])
                nc.vector.tensor_mul(xs[:], xs[:], sin_sb[:, sj])
                nc.vector.tensor_add(dst[h][:, sj], dst[h][:, sj], xs[:])

            def rope_mul(dst, i, h, m):
                """apply rstd (per-column broadcast) in place."""
                sj = slice(m * 1024, (m + 1) * 1024)
                nc.vector.tensor_mul(dst[h][:, sj], dst[h][:, sj], rbc[i][:, sj])

            # ---------- phase 1: QK projections + sumsq, ARs per half ------
            with ExitStack() as ph1:
                wpool = ph1.enter_context(tc.tile_pool(name=P("wpool"), bufs=1))
                hid = ph1.enter_context(tc.tile_pool(name=P("hid"), bufs=1))
                wq_sb = wpool.tile([128, KT, DC], BF16)
                wk_sb = wpool.tile([128, KT, DC], BF16)
                wv_sb = wpool.tile([128, KT, DC], BF16)
                nc.sync.dma_start(wq_sb[:], wqT[:])
                nc.sync.dma_start(wk_sb[:], wkT[:])
                nc.sync.dma_start(wv_sb[:], wvT[:])

                hch = []
                for j in range(NCH):
                    sj = slice(j * 512, (j + 1) * 512)
                    hc = hid.tile([128, KT, 512], BF16, name=f"hch{j}")
                    nc.sync.dma_start(hc[:], hidT[j])
                    hch.append(hc)

                    for (wsb, raw, bias) in ((wq_sb, qraw, bq_sb), (wk_sb, kraw, bk_sb)):
                        for h in range(HPC):
                            pq = psA.tile([128, 512], F32, name="pqk")
                            for t in range(KT):
                                nc.tensor.matmul(
                                    pq[:], wsb[:, t, h * DH:(h + 1) * DH],
                                    hc[:, t, :], start=(t == 0), stop=(t == KT - 1))
                            nc.scalar.activation(raw[h][:, sj], pq[:], AF.Identity,
                                                 bias=bias[:, h:h + 1])

                    # partial sum-of-squares (un-weighted: scale by 1/nw^2)
                    for idx, (raw, inv2) in ((0, (qraw, inq_sb)), (1, (kraw, ink_sb))):
                        pss = psA.tile([1, 512], F32, name="pss")
                        for h in range(HPC):
                            sq = rwork.tile([128, 512], BF16, name="sq")
                            nc.vector.scalar_tensor_tensor(
                                sq[:], raw[h][:, sj], inv2[:, h:h + 1],
                                raw[h][:, sj], ALU.mult, ALU.mult)
                            nc.tensor.matmul(pss[:], ones_col[:], sq[:],
                                             start=(h == 0), stop=(h == HPC - 1))
                        ssv = rwork.tile([1, 512], F32, name="ssv")
                        nc.vector.tensor_copy(ssv[:], pss[:])
                        m, off = j // 2, (j % 2) * 512
                        nc.sync.dma_start(
                            ar_in[m][:, idx * 1024 + off: idx * 1024 + off + 512],
                            ssv[:])

                    if (j == 1 or j == NCH - 1) and stage >= 2:
                        m = j // 2
                        if ag_mode != 'nocoll':
                            nc.gpsimd.collective_compute(
                                "AllReduce", ALU.add, replica_groups=rg,
                                ins=[ar_in[m][:].opt()], outs=[ar_out[m][:].opt()])
                        else:
                            nc.gpsimd.dma_start(ar_out[m][:], ar_in[m][:])
                    if j == 1 and stage >= 2:
                        # k rope for half 0 on DVE while PE continues QK
                        rope_rot(kraw, kTt, 0, 0)
                        rope_rot(kraw, kTt, 1, 0)

                # ---- v phase on PE; rope + rstd prep on DVE/ACT/Pool ----
                for j in range(NCH):
                    for st in range(4):
                        gst = j * 4 + st
                        pvp = psA.tile([128, DC], F32, name="pvp")
                        for t in range(KT):
                            nc.tensor.matmul(
                                pvp[:], hch[j][:, t, st * 128:(st + 1) * 128],
                                wv_sb[:, t, :], start=(t == 0), stop=False)
                        nc.tensor.matmul(pvp[:], ones_row[:], bv_sb[:],
                                         start=False, stop=True)
                        if gst == SP // 128 - 1:
                            nc.vector.memset(v_sb[:, gst, :], 0.0)
                            nv = S - (SP // 128 - 1) * 128
                            nc.scalar.activation(v_sb[0:nv, gst, :], pvp[0:nv, :],
                                                 AF.Copy)
                        else:
                            nc.scalar.activation(v_sb[:, gst, :], pvp[:], AF.Copy)

                if stage >= 2:
                    rope_rot(qraw, qT, 0, 0)
                    rope_rot(qraw, qT, 1, 0)
                    rstd_prep(0)
                    for h in range(HPC):
                        rope_mul(kTt, 1, h, 0)
                        rope_mul(qT, 0, h, 0)
                    rope_rot(kraw, kTt, 0, 1)
                    rope_rot(kraw, kTt, 1, 1)
                    rope_rot(qraw, qT, 0, 1)
                    rope_rot(qraw, qT, 1, 1)
                    rstd_prep(1)
                    for h in range(HPC):
                        rope_mul(kTt, 1, h, 1)
                        nc.vector.memset(kTt[h][:, S:SP], 0.0)
                    for h in range(HPC):
                        rope_mul(qT, 0, h, 1)

        if stage < 3:
            return
        # ---------- attention, chunked AllGather, projection ----
        with ExitStack() as ph4:
            aw = ph4.enter_context(tc.tile_pool(name=P("aw"), bufs=2))
            oT = [aw.tile([128, SP], BF16, name=f"oT{h}_{rep}", bufs=1)
                  for h in range(HPC)]
            expp = ph4.enter_context(tc.tile_pool(name=P("expp"), bufs=3))
            denp = ph4.enter_context(tc.tile_pool(name=P("denp"), bufs=2))
            psC = ph4.enter_context(tc.tile_pool(name=P("psC"), bufs=2, space="PSUM"))
            psPV = ph4.enter_context(tc.tile_pool(name=P("psPV"), bufs=2, space="PSUM"))
            psM = ph4.enter_context(tc.tile_pool(name=P("psM"), bufs=2, space="PSUM"))

            wo_sb = aw.tile([128, KT, DC], BF16, bufs=1)
            nc.sync.dma_start(wo_sb[:], woT[:])

            agos = []

            def project(j):
                ago = agos[j]
                och = aw.tile([128, KT, 512], BF16, name="och")
                for q4 in range(4):
                    nc.gpsimd.dma_start(
                        och[:, q4 * 4:(q4 + 1) * 4, :],
                        ago[q4 * 512:(q4 + 1) * 512, :]
                        .rearrange("(t p) s -> p t s", p=128))
                for h in range(HPC):
                    pout = psM.tile([128, 512], F32, name="psm")
                    for t in range(KT):
                        nc.tensor.matmul(
                            pout[:], wo_sb[:, t, h * DH:(h + 1) * DH],
                            och[:, t, :], start=(t == 0), stop=(t == KT - 1))
                    ot = aw.tile([128, 512], F32, name="ot")
                    nc.scalar.activation(ot[:], pout[:], AF.Identity,
                                         bias=bo_sb[:, h:h + 1])
                    w = min(512, S - j * 512)
                    nc.sync.dma_start(
                        outT[h * DH:(h + 1) * DH, j * 512:j * 512 + w],
                        ot[:, 0:w])

            for j in range(NCH):
                sj = slice(j * 512, (j + 1) * 512)
                for h in range(HPC):
                    po = psPV.tile([128, 512], F32, name="pv")
                    # software pipeline: scores one kt-pair ahead of PV
                    pscores = []
                    partials = []   # binomial tree of bf16 exp-tile sums

                    def scores(kp):
                        ps = psC.tile([128, 1024], F32, name="psc")
                        nc.tensor.matmul(
                            ps[:, 0:512], kTt[h][:, kp * 256:kp * 256 + 128],
                            qT[h][:, sj], start=True, stop=True)
                        nc.tensor.matmul(
                            ps[:, 512:1024], kTt[h][:, kp * 256 + 128:kp * 256 + 256],
                            qT[h][:, sj], start=True, stop=True)
                        pscores.append(ps)

                    def exp_pv(kp):
                        ps = pscores[kp]
                        et = expp.tile([128, 1024], BF16, name="et")
                        nc.scalar.activation(et[:], ps[:], AF.Exp,
                                             scale=inv_sqrt_dh)
                        nc.tensor.matmul(
                            po[:], v_sb[:, 2 * kp, h * DH:(h + 1) * DH],
                            et[:, 0:512], start=(kp == 0), stop=False)
                        nc.tensor.matmul(
                            po[:], v_sb[:, 2 * kp + 1, h * DH:(h + 1) * DH],
                            et[:, 512:1024], start=False, stop=(kp == 7))
                        # denominator partials on DVE (bf16 binomial tree)
                        cur, rank = et, 0
                        while partials and partials[-1][1] == rank:
                            prev, _ = partials.pop()
                            dst = denp.tile([128, 1024], BF16, name=f"den{rank}")
                            nc.vector.tensor_add(dst[:], prev[:], cur[:])
                            cur, rank = dst, rank + 1
                        partials.append((cur, rank))

                    scores(0)
                    for kp in range(8):
                        if kp + 1 < 8:
                            scores(kp + 1)
                        exp_pv(kp)
                    acc = partials[0][0]
                    psm = psM.tile([128, 512], F32, name="psm")
                    nc.tensor.matmul(psm[:], ones_sq[:], acc[:, 0:512],
                                     start=True, stop=False)
                    nc.tensor.matmul(psm[:], ones_sq[:], acc[:, 512:1024],
                                     start=False, stop=True)
                    rec = aw.tile([128, 512], F32, name="rec")
                    nc.vector.reciprocal(rec[:], psm[:])
                    nc.vector.tensor_mul(oT[h][:, sj], po[:], rec[:])

                # issue this chunk's AllGather right away; it overlaps the
                # attention of the remaining chunks
                agi = dram.tile([DC, 512], BF16, name=f"agi{j}_{rep}")
                ago = dram.tile([DIM, 512], BF16, addr_space="Shared",
                                name=f"ago{j}_{rep}")
                for h in range(HPC):
                    nc.sync.dma_start(agi[h * DH:(h + 1) * DH, :], oT[h][:, sj])
                if ag_mode == 'chunk4':
                    nc.gpsimd.collective_compute(
                        "AllGather", ALU.bypass, replica_groups=rg,
                        ins=[agi[:].opt()], outs=[ago[:].opt()])
                agos.append(ago)
                if stage >= 4 and j >= 2:
                    project(j - 2)

            if stage >= 4:
                project(NCH - 2)
                project(NCH - 1)

    with tile.TileContext(nc) as tc, \
            nc.allow_low_precision(reason="bf16 softmax path validated vs ref"):
        for rep in range(repeat):
            with ExitStack() as top:
                emit(tc, top, rep)

    nc.compile()
    return nc


def _prep_inputs(hidden_states, freqs_cos, freqs_sin, wq, bq, wk, bk, wv, bv,
                 norm_q_w, norm_k_w, wo, bo):
    """Host-side shard + layout prep. Returns in_maps for 8 cores."""
    f32 = np.float32
    import ml_dtypes
    bf16 = ml_dtypes.bfloat16

    hid = np.asarray(hidden_states)[0].T.astype(f32)
    hidT = np.zeros((DIM, SP), dtype=f32)
    hidT[:, :S] = hid
    # pre-tile to [chunk j, partition p, ktile t, col c]: d = t*128+p, s = j*512+c
    hidT = np.ascontiguousarray(
        hidT.reshape(KT, 128, SP // 512, 512).transpose(2, 1, 0, 3)).astype(bf16)

    def tile_w(wT):                       # [DIM, DC] -> [128, KT, DC]
        return np.ascontiguousarray(
            wT.reshape(KT, 128, DC).transpose(1, 0, 2)).astype(bf16)

    # RoPE tables: c_j[s] = cos[0,s,0,2j], s_j[s] = sin[0,s,0,2j+1]; stack [t;t]
    c = np.asarray(freqs_cos)[0, :, 0, 0::2].astype(f32).T          # [64, S]
    s = np.asarray(freqs_sin)[0, :, 0, 1::2].astype(f32).T          # [64, S]
    cosT = np.zeros((DH, SP), dtype=f32)
    sinT = np.zeros((DH, SP), dtype=f32)
    cosT[0:64, :S] = c
    cosT[64:128, :S] = c
    sinT[0:64, :S] = -s
    sinT[64:128, :S] = s
    cosT = cosT.astype(bf16)
    sinT = sinT.astype(bf16)

    perm = np.concatenate([np.arange(0, DH, 2), np.arange(1, DH, 2)])
    wq = np.asarray(wq)
    wk = np.asarray(wk)
    wv = np.asarray(wv)
    wo = np.asarray(wo)
    bqv = np.asarray(bq)
    bkv = np.asarray(bk)
    bvv = np.asarray(bv)
    bov = np.asarray(bo)
    nq = np.asarray(norm_q_w)
    nk = np.asarray(norm_k_w)

    in_maps = []
    for core in range(N_CORES):
        rows = slice(core * DC, (core + 1) * DC)

        def permuted(mat_rows):                                     # [DC, DIM]
            blocks = [mat_rows[h * DH:(h + 1) * DH][perm] for h in range(HPC)]
            return np.concatenate(blocks, axis=0)

        def permuted_vec(vec_rows):                                 # [HPC, DH]
            blocks = [vec_rows[h * DH:(h + 1) * DH][perm] for h in range(HPC)]
            return np.stack(blocks, axis=0)

        # fold the norm weight into wq/wk and bq/bk (rows scaled by nw)
        nq_p = permuted_vec(nq[rows].astype(f32))                   # [HPC, DH]
        nk_p = permuted_vec(nk[rows].astype(f32))
        wq_c = permuted(wq[rows].astype(f32)) * nq_p.reshape(DC, 1)
        wk_c = permuted(wk[rows].astype(f32)) * nk_p.reshape(DC, 1)
        bq_c = permuted_vec(bqv[rows].astype(f32)) * nq_p
        bk_c = permuted_vec(bkv[rows].astype(f32)) * nk_p

        in_maps.append({
            "hidT": hidT,
            "wqT": tile_w(np.ascontiguousarray(wq_c.T)),
            "wkT": tile_w(np.ascontiguousarray(wk_c.T)),
            "wvT": tile_w(np.ascontiguousarray(wv[rows].astype(f32).T)),
            "woT": tile_w(np.ascontiguousarray(wo[rows].astype(f32).T)),
            "cosT": cosT,
            "sinT": sinT,
            "bq": bq_c,
            "bk": bk_c,
            "bv": bvv[rows].astype(bf16).reshape(1, DC),
            "bo": bov[rows].astype(f32).reshape(HPC, DH),
            "inwq": 1.0 / (nq_p * nq_p),
            "inwk": 1.0 / (nk_p * nk_p),
        })
    return in_maps


def kernel(**inputs):
    global _COMPILED
    if _COMPILED is None:
        _COMPILED = _build()
    nc = _COMPILED
    in_maps = _prep_inputs(**inputs)
    res = run_bass_kernel_spmd(nc, in_maps, core_ids=list(range(N_CORES)))
    out = np.empty((1, S, DIM), dtype=np.float32)
    for core in range(N_CORES):
        out[0, :, core * DC:(core + 1) * DC] = res.results[core]["outT"].T
    return out
